# revision 24
# baseline (speedup 1.0000x reference)
"""Trainium2 Bass kernel for a prenorm transformer Block (B=8, N=1024, D=768,
12 heads, MLP hidden 3072), data-parallel over batch across 8 NeuronCores.

Layout strategy: activations live transposed on-device — features on SBUF
partitions, tokens on the free dimension — so the whole chain
(QKV -> attention -> proj -> LN -> MLP -> LN) feeds the PE without any
on-device transposes:

  - qT/kT per head land as [64 dims (partitions), 1024 tokens]; scores are
    computed transposed (scoresT[m, n] = k_m . q_n) so softmax's exp is a
    plain ACT pass; the denominators come out of the attn@v matmul via an
    extra ones-column on the stationary V operand.
  - Softmax skips max-subtraction: scores here are bounded (|s| < ~4), exp
    cannot overflow fp32, and softmax is shift-invariant so results match.
  - The per-pair softmax division runs entirely off the PE critical path:
    the PSUM context tiles are copied to SBUF right away (freeing the PSUM
    banks for the next pair), then reciprocal_approx_fast + a DRAM-roundtrip
    partition-broadcast + one multiply produce ctx while the next pair's
    matmuls/exps stream. ctx is only consumed by proj at the end.
  - LayerNorm: sums and sum-of-squares reduce over features (partitions) on
    the PE as ones-vector matmuls; the per-token scale/shift expand to
    [128, 512] tiles as PE outer products against stationary gamma/beta
    rows, so the affine is 2 fused DVE ops per feature chunk. Squares run
    on the Scalar engine (tableless), 1/std via Sqrt + recip_approx_fast.
    Each LN is emitted as separate stats/affine halves woven between the
    surrounding matmul phases so the PE never drains.
  - MLP fc2 accumulates all 24 hidden chunks of a token half in PSUM
    (no vector-add accumulation); both residual adds are single fused
    scalar_tensor_tensor ops reading the PSUM result directly.
  - All matmuls use f16 weights / f16 or float32r moving operands
    (1 cycle/row on the PE); all weights are prefetched to SBUF during the
    ACT-bound attention phase.

Host side pre-transposes x and all weights, folds the attention scale into
the Q columns of w_qkv, and transposes the final output back.
"""
import sys
import types

sys.path.insert(0, "/opt/trn_rl_repo")

# concourse.bass_utils imports antenv.axon_hooks when tracing is requested;
# provide a no-op registry if the container image lacks that module so a
# BASS_TRACE=1 environment degrades to "no trace" instead of crashing.
try:
    import antenv.axon_hooks  # noqa: F401
except Exception:
    try:
        import antenv

        _hooks = types.ModuleType("antenv.axon_hooks")
        _hooks._hook = None

        def _set_hook(h):
            _hooks._hook = h

        def _get_hook():
            return _hooks._hook

        _hooks.set_axon_ntff_profile_hook = _set_hook
        _hooks.get_axon_ntff_profile_hook = _get_hook
        sys.modules["antenv.axon_hooks"] = _hooks
        antenv.axon_hooks = _hooks
    except Exception:
        pass

import numpy as np

import concourse.bass as bass
import concourse.tile as tile
from concourse import mybir
from concourse.bass_utils import run_bass_kernel_spmd

F32R = mybir.dt.float32r
F32 = mybir.dt.float32
F16 = mybir.dt.float16
AF = mybir.ActivationFunctionType
OP = mybir.AluOpType
I16 = mybir.dt.int16
I32 = mybir.dt.int32
K16 = 0x7799            # f16 reciprocal magic + 1  (seed = K - bits(d))
K32 = 0x7EF311C4        # f32 reciprocal magic + 1

NCORES = 8
D, HEADS, HID, N = 768, 12, 3072, 1024
HD = D // HEADS                  # 64 head dim
DC = D // 128                    # 6 feature chunks
NB = N // 512                    # 2 moving-dim blocks
MT = N // 128                    # 8 token tiles
FT = HID // 128                  # 24 hidden chunks
EPS = 1e-6

LAST_RESULT = None               # BassKernelResults of the most recent run


# The walrus build in this container rejects instructions carrying more than
# a couple of sync waits ("Too many sync wait commands"); self-loading fp32r
# matmuls reject more than one. Excess waits are hoisted onto standalone
# EventSemaphore carriers placed right before the instruction on the same
# engine, which is semantically identical (waits gate the engine stream).
_MM_OPS = ("Matmult", "Ldweights")


def _split_excess_waits(nc, default_limit=1, matmul_limit=0):
    counter = 0
    for f in nc.m.functions:
        for bb in f.blocks:
            new_insts = []
            for inst in bb.instructions:
                si = inst.sync_info
                waits = list(si.on_wait) if si and si.on_wait else []
                limit = matmul_limit if inst.opcode in _MM_OPS else default_limit
                if len(waits) > limit:
                    keep, move = waits[:limit], waits[limit:]
                    for w in move:
                        counter += 1
                        ev = mybir.InstEventSemaphore(
                            name=f"I-waitsplit-{counter}",
                            engine=inst.engine,
                            sync_info=mybir.SyncInfo(on_wait=[w], on_update=[]),
                        )
                        nc.register_instruction(ev, overwrite=True)
                        new_insts.append(ev)
                    inst.sync_info = mybir.SyncInfo(
                        on_wait=keep, on_update=list(si.on_update) if si else []
                    )
                new_insts.append(inst)
            bb.instructions = new_insts
    return counter


def _build():
    nc = bass.Bass()

    xT16 = nc.dram_tensor("xT16", [D, N], F16, kind="ExternalInput")
    wqkvT = nc.dram_tensor("wqkvT", [D, 3 * D], F16, kind="ExternalInput")
    wprojT = nc.dram_tensor("wprojT", [D, D], F16, kind="ExternalInput")
    wfc1T = nc.dram_tensor("wfc1T", [D, HID], F16, kind="ExternalInput")
    wfc2T = nc.dram_tensor("wfc2T", [HID, D], F16, kind="ExternalInput")
    bprojC = nc.dram_tensor("bprojC", [128, DC], F32, kind="ExternalInput")
    bfc1C = nc.dram_tensor("bfc1C", [128, FT], F32, kind="ExternalInput")
    bfc2C = nc.dram_tensor("bfc2C", [128, DC], F32, kind="ExternalInput")
    gamma1C = nc.dram_tensor("gamma1C", [128, DC], F32, kind="ExternalInput")
    gamma2C = nc.dram_tensor("gamma2C", [128, DC], F32, kind="ExternalInput")
    gb1R = nc.dram_tensor("gb1R", [2, D], F32, kind="ExternalInput")
    gb2R = nc.dram_tensor("gb2R", [2, D], F32, kind="ExternalInput")
    yT = nc.dram_tensor("yT", [D, N], F32, kind="ExternalOutput")

    with tile.TileContext(nc) as tc:
        const = tc.alloc_tile_pool(name="const", bufs=1)
        stats = tc.alloc_tile_pool(name="stats", bufs=1)
        dscr = tc.alloc_tile_pool(name="dscr", bufs=4, space="DRAM")

        ones128 = const.tile([128, 1], F32R)
        nc.vector.tensor_copy(ones128[:], nc.const_aps.tensor(1.0, (128, 1)))
        ones_row = const.tile([1, 128], F32R)
        nc.vector.tensor_copy(ones_row[:], nc.const_aps.tensor(1.0, (1, 128)))
        eps_t = const.tile([1, 1], F32)
        nc.vector.memset(eps_t[:], EPS)
        bproj_sb = const.tile([128, DC], F32)
        bfc1_sb = const.tile([128, FT], F32)
        bfc2_sb = const.tile([128, DC], F32)
        g1_sb = const.tile([128, DC], F32)
        g2_sb = const.tile([128, DC], F32)
        gbf_sb = const.tile([2, D], F32)
        gb1_sb = const.tile([2, D], F16)
        gb2_sb = const.tile([2, D], F16)
        for t, src in ((bproj_sb, bprojC), (bfc1_sb, bfc1C), (bfc2_sb, bfc2C),
                       (g1_sb, gamma1C), (g2_sb, gamma2C)):
            nc.sync.dma_start(out=t[:], in_=src[:])
        nc.sync.dma_start(out=gbf_sb[:], in_=gb1R[:])
        nc.vector.tensor_copy(gb1_sb[:], gbf_sb[:])
        nc.sync.dma_start(out=gbf_sb[:], in_=gb2R[:])
        nc.vector.tensor_copy(gb2_sb[:], gbf_sb[:])
        # moving operand for the LN shift outer-product: row0 = -mu/std
        # (written per LN half), row1 = ones
        m2 = [const.tile([2, 512], F16, name=f"m2_{nb}") for nb in range(NB)]
        for t in m2:
            nc.vector.memset(t[:], 1.0)   # row 0 is rewritten per LN half

        def ln_stats(src_sb, nb, p_t, ps_ln):
            """Per-token 1/std (r) and -mu/std (m2[nb] row 0) for token half
            nb of src_sb [128, DC, N] f32r. Reductions over features run on
            the PE; the small chains on ACT/DVE."""
            sl = slice(nb * 512, nb * 512 + 512)
            s1 = ps_ln.tile([1, 512], F32, tag="s1", name="s1")
            s2 = ps_ln.tile([1, 512], F32, tag="s2", name="s2")
            for c in range(DC):
                nc.tensor.matmul(s1[:], ones128[:], src_sb[:, c, sl],
                                 start=(c == 0), stop=(c == DC - 1))
            for c in range(DC):
                sq = p_t.tile([128, 512], F32R, tag="sq", name="sq")
                nc.scalar.activation(out=sq[:], in_=src_sb[:, c, sl].bitcast(F32),
                                     func=AF.Square)
                nc.tensor.matmul(s2[:], ones128[:], sq[:],
                                 start=(c == 0), stop=(c == DC - 1))
            u = stats.tile([1, 512], F32, tag=f"u{nb}", name="u")
            w = stats.tile([1, 512], F32, tag=f"w{nb}", name="w")
            sd = stats.tile([1, 512], F32R, tag=f"sd{nb}", name="sd")
            nc.scalar.activation(out=u[:], in_=s1[:], func=AF.Square)
            nc.vector.scalar_tensor_tensor(out=w[:], in0=s2[:], scalar=float(D),
                                           in1=u[:], op0=OP.mult, op1=OP.subtract)
            nc.scalar.activation(out=sd[:], in_=w[:], func=AF.Sqrt,
                                 bias=eps_t[:], scale=1.0 / (D * D))   # std, f32r
            return s1, sd

        def ln_affine(src_sb, gamcol, gbrows, nb, stats_t, out_full, out_dt,
                      p_t, ps_ln, ps_b, dma_out=None):
            """out = src * gamma * bcast(1/std) + (gamma (x) m2row0 + beta).
            1/std via magic-seed + 2 Newton steps on DVE (this walrus build
            rejects custom-DVE and ACT Rsqrt/Reciprocal); the broadcast is a
            PE outer product; per chunk one fused STT plus one TT add on
            DVE. out_full is a [128, DC, N] tile or None (then per-chunk
            ring tiles are DMAed straight out)."""
            s1, sd = stats_t
            sl = slice(nb * 512, nb * 512 + 512)
            na = stats.tile([1, 512], F32, tag="na", name="na")
            nb_ = stats.tile([1, 512], F32, tag="nb", name="nb")
            ng = stats.tile([1, 512], F32, tag="ng", name="ng")
            rT = stats.tile([1, 512], F32R, tag="rT", name="rT")
            sdf = sd[:].bitcast(F32)
            nc.vector.tensor_scalar(out=na[:].bitcast(I32), in0=sd[:].bitcast(I32),
                                    scalar1=-1, scalar2=None, op0=OP.bitwise_xor)
            nc.vector.tensor_scalar(out=nb_[:].bitcast(I32), in0=na[:].bitcast(I32),
                                    scalar1=K32, scalar2=None, op0=OP.add)
            nc.vector.tensor_mul(na[:], sdf, nb_[:])
            nc.vector.tensor_scalar(out=ng[:], in0=na[:], scalar1=-1.0,
                                    scalar2=2.0, op0=OP.mult, op1=OP.add)
            nc.vector.tensor_mul(na[:], nb_[:], ng[:])        # y1
            nc.vector.tensor_mul(nb_[:], sdf, na[:])
            nc.vector.tensor_scalar(out=ng[:], in0=nb_[:], scalar1=-1.0,
                                    scalar2=2.0, op0=OP.mult, op1=OP.add)
            nc.vector.tensor_mul(rT[:], na[:], ng[:])         # 1/std, f32r
            nc.vector.scalar_tensor_tensor(out=m2[nb][0:1, :], in0=s1[:],
                                           scalar=-1.0 / D, in1=rT[:].bitcast(F32),
                                           op0=OP.mult, op1=OP.mult)  # -mu/std
            R = ps_ln.tile([128, 512], F32, tag="R", name="R")
            nc.tensor.matmul(R[:], ones_row[:], rT[:],
                             start=True, stop=True)          # bcast 1/std
            for c in range(DC):
                B = ps_b.tile([128, 512], F32, tag="B", name="B")
                nc.tensor.matmul(B[:], gbrows[:, c * 128:(c + 1) * 128],
                                 m2[nb][:], start=True, stop=True)
                t = p_t.tile([128, 512], F32, tag="t", name="t")
                nc.vector.scalar_tensor_tensor(
                    out=t[:], in0=src_sb[:, c, sl].bitcast(F32),
                    scalar=gamcol[:, c:c + 1], in1=R[:],
                    op0=OP.mult, op1=OP.mult)
                if out_full is not None:
                    nc.vector.tensor_add(out_full[:, c, sl], t[:], B[:])
                else:
                    o = p_t.tile([128, 512], out_dt, tag="o", name="o")
                    nc.vector.tensor_add(o[:], t[:], B[:])
                    nc.sync.dma_start(out=dma_out[c * 128:(c + 1) * 128, sl],
                                      in_=o[:])

        # ---------------- Phase 1: QKV projections ----------------
        p_w1 = tc.alloc_tile_pool(name="p_w1", bufs=1)
        w1_sb = p_w1.tile([128, DC, HID], F16)
        p_w2 = tc.alloc_tile_pool(name="p_w2", bufs=1)
        w2_sb = p_w2.tile([128, FT, D], F16)
        p_xT16 = tc.alloc_tile_pool(name="p_xT16", bufs=1, side="right")
        xT16_sb = p_xT16.tile([128, DC, N], F16)
        def dma_x16(nb):
            sl = slice(nb * 512, nb * 512 + 512)
            for c in range(DC):
                nc.sync.dma_start(out=xT16_sb[:, c, sl],
                                  in_=xT16[c * 128:(c + 1) * 128, sl])
        dma_x16(0)
        p_wproj = tc.alloc_tile_pool(name="p_wproj", bufs=1, side="right")
        wproj_sb = p_wproj.tile([128, DC, D], F16)
        p_ctx = tc.alloc_tile_pool(name="p_ctx", bufs=1, side="right")
        ctx_sb = p_ctx.tile([128, DC, N], F16)
        p_qk = tc.alloc_tile_pool(name="p_qk", bufs=1, side="right")
        p_v = tc.alloc_tile_pool(name="p_v", bufs=1, side="right")
        p_attn = tc.alloc_tile_pool(name="p_attn", bufs=4, side="right")
        p_wqkv = tc.alloc_tile_pool(name="p_wqkv", bufs=1, side="right")
        wqkv_sb = p_wqkv.tile([128, DC, 3 * D], F16)
        # weight columns arrive in matmul order: (q_i, k_i) pairs first
        # fine-grained, then the second x half, then v as one block
        def dma_wcols(jts):
            for jt in jts:
                for c in range(DC):
                    cs = slice(jt * 128, (jt + 1) * 128)
                    nc.sync.dma_start(out=wqkv_sb[:, c, cs],
                                      in_=wqkvT[c * 128:(c + 1) * 128, cs])
        dma_wcols([0, DC])
        dma_x16(1)
        dma_wcols([1, DC + 1, 2, DC + 2])
        dma_wcols([3, DC + 3, 4, DC + 4, 5, DC + 5])
        for c in range(DC):
            nc.sync.dma_start(out=wqkv_sb[:, c, 2 * D:3 * D],
                              in_=wqkvT[c * 128:(c + 1) * 128, 2 * D:3 * D])
        q_sb = p_qk.tile([128, DC, N], F16)
        k_sb = p_qk.tile([128, DC, N], F16)
        v_sb = p_v.tile([128, MT, HEADS, HD + 1], F16)
        nc.vector.tensor_copy(v_sb[:, :, :, HD:HD + 1],
                              nc.const_aps.tensor(1.0, (128, MT, HEADS, 1)))

        ps2s = tc.alloc_tile_pool(name="ps2s", bufs=1, space="PSUM")
        ps1 = tc.alloc_tile_pool(name="ps1", bufs=2, space="PSUM")
        ps1v = tc.alloc_tile_pool(name="ps1v", bufs=1, space="PSUM")
        # warm the Exp activation table while QKV streams, so the first
        # attention exp pays no table load
        warm = stats.tile([1, 64], F32, tag="warm", name="warm")
        nc.scalar.activation(out=warm[:], in_=warm[:], func=AF.Exp)

        def qk_chain(jt, nb):
            sl = slice(nb * 512, nb * 512 + 512)
            ps = ps1.tile([128, 512], F32, tag="qk", name="psqk")
            for c in range(DC):
                nc.tensor.matmul(ps[:], wqkv_sb[:, c, jt * 128:(jt + 1) * 128],
                                 xT16_sb[:, c, sl],
                                 start=(c == 0), stop=(c == DC - 1))
            dst = q_sb if jt < DC else k_sb
            nc.scalar.activation(out=dst[:, jt % DC, sl], in_=ps[:],
                                 func=AF.Copy, scale=1.0)

        def v_chain(mt):
            ps = ps1v.tile([128, D], F32, tag="v", name="psv")
            for c in range(DC):
                nc.tensor.matmul(ps[:, 0:512],
                                 xT16_sb[:, c, mt * 128:(mt + 1) * 128],
                                 wqkv_sb[:, c, 2 * D:2 * D + 512],
                                 start=(c == 0), stop=(c == DC - 1))
                nc.tensor.matmul(ps[:, 512:768],
                                 xT16_sb[:, c, mt * 128:(mt + 1) * 128],
                                 wqkv_sb[:, c, 2 * D + 512:3 * D],
                                 start=(c == 0), stop=(c == DC - 1))
            nc.vector.tensor_copy(v_sb[:, mt, :, 0:HD],
                                  ps[:].rearrange("p (h d) -> p h d", h=HEADS))

        def scores_mt(pr, mt):
            pse = ps2s.tile([128, N], F32, tag="se", name="pse")
            pso = ps2s.tile([128, N], F32, tag="so", name="pso")
            msl = slice(mt * 128, mt * 128 + 128)
            for nb in range(NB):
                sl = slice(nb * 512, nb * 512 + 512)
                nc.tensor.matmul(pse[:, sl], k_sb[0:64, pr, msl],
                                 q_sb[0:64, pr, sl], start=True, stop=True)
                nc.tensor.matmul(pso[:, sl], k_sb[64:128, pr, msl],
                                 q_sb[64:128, pr, sl], start=True, stop=True)
            ae = p_attn.tile([128, N], F16, tag="attnT", name="ae")
            ao = p_attn.tile([128, N], F16, tag="attnT", name="ao")
            nc.scalar.activation(out=ae[:], in_=pse[:], func=AF.Exp)
            nc.scalar.activation(out=ao[:], in_=pso[:], func=AF.Exp)
            return ae, ao

        for jt in (0, DC):
            qk_chain(jt, 0)
        for jt in (0, DC):
            qk_chain(jt, 1)
        for pair in range(1, DC):
            qk_chain(pair, 0)
            qk_chain(DC + pair, 0)
            qk_chain(pair, 1)
            qk_chain(DC + pair, 1)
        # prerun pair-0 scores/exps between v chains: the ACT pipeline gets a
        # head start and the PE has no idle seam entering attention
        v_chain(0)
        pre = [scores_mt(0, 0)]
        for mt in range(1, MT):
            v_chain(mt)
            if mt == 1:
                pre.append(scores_mt(0, 1))
        ps1v.release()
        ps1.release()
        p_wqkv.release()
        p_div = tc.alloc_tile_pool(name="p_div", bufs=2, side="right")
        p_div1 = tc.alloc_tile_pool(name="p_div1", bufs=1, side="right")

        # proj/MLP weights: DMA during the (ACT-bound) attention phase
        for c in range(DC):
            nc.sync.dma_start(out=wproj_sb[:, c, :], in_=wprojT[c * 128:(c + 1) * 128, :])
        for c in range(DC):
            nc.sync.dma_start(out=w1_sb[:, c, :], in_=wfc1T[c * 128:(c + 1) * 128, :])
        for fc in range(FT):
            nc.sync.dma_start(out=w2_sb[:, fc, :], in_=wfc2T[fc * 128:(fc + 1) * 128, :])

        # ---------------- Phase 2: attention (head pairs) ----------------
        ps2c = tc.alloc_tile_pool(name="ps2c", bufs=1, space="PSUM")

        for pr in range(HEADS // 2):
            cps = {}
            for j in range(4):          # j = h01*2 + nb
                cps[j] = ps2c.tile([HD + 1, 512], F32, tag=f"c{j}", name=f"cps{j}")
            for mt in range(MT):
                if pr == 0 and mt < len(pre):
                    ae, ao = pre[mt]
                else:
                    ae, ao = scores_mt(pr, mt)
                for h01, at_t in ((0, ae), (1, ao)):
                    h = 2 * pr + h01
                    for nb in range(NB):
                        sl = slice(nb * 512, nb * 512 + 512)
                        nc.tensor.matmul(cps[2 * h01 + nb][:], v_sb[:, mt, h, :],
                                         at_t[:, sl],
                                         start=(mt == 0), stop=(mt == MT - 1))
            # softmax division — the f16 copies free the cps PSUM banks for
            # the next pair; the Newton reciprocal + DRAM-roundtrip
            # broadcast + multiply are off the PE critical path (ctx is only
            # consumed by proj after the last pair).
            craw = p_div.tile([HD, 4 * 512], F16, tag="craw", name="craw")
            dnt = p_div1.tile([1, 4 * 512], F16, tag="dn", name="dnt")
            for j in range(4):
                nc.vector.tensor_copy(craw[:, j * 512:(j + 1) * 512],
                                      cps[j][0:HD, :])
                nc.vector.tensor_copy(dnt[0:1, j * 512:(j + 1) * 512],
                                      cps[j][HD:HD + 1, :])
            dn = dnt[:]
            a = p_div1.tile([1, 4 * 512], F16, tag="a", name="na")
            b = p_div1.tile([1, 4 * 512], F16, tag="b", name="nb")
            g = p_div1.tile([1, 4 * 512], F16, tag="g", name="ng")
            nc.vector.tensor_scalar(out=a[:].bitcast(I16), in0=dn.bitcast(I16),
                                    scalar1=-1, scalar2=None, op0=OP.bitwise_xor)
            nc.vector.tensor_scalar(out=b[:].bitcast(I16), in0=a[:].bitcast(I16),
                                    scalar1=K16, scalar2=None, op0=OP.add)
            nc.vector.tensor_mul(a[:], dn, b[:])
            nc.vector.tensor_scalar(out=g[:], in0=a[:], scalar1=-1.0,
                                    scalar2=2.0, op0=OP.mult, op1=OP.add)
            nc.vector.tensor_mul(a[:], b[:], g[:])            # y1
            if pr < HEADS // 2 - 1:
                nc.vector.tensor_mul(b[:], dn, a[:])
                nc.vector.tensor_scalar(out=g[:], in0=b[:], scalar1=-1.0,
                                        scalar2=2.0, op0=OP.mult, op1=OP.add)
                nc.vector.tensor_mul(b[:], a[:], g[:])        # 1/denom
            else:
                # last pair: 1 Newton step (0.26% worst case) — this division
                # gates proj, shave its latency
                a, b = b, a
            scr = dscr.tile([4 * 512], F16, name="bscr")
            nc.sync.dma_start(out=scr[:], in_=b[:])
            recb = p_div1.tile([128, 4 * 512], F16, tag="recb", name="recb")
            nc.sync.dma_start(out=recb[:],
                              in_=scr[:].unsqueeze(0).to_broadcast([128, 4 * 512]))
            for h01 in range(2):
                half = h01 * 64
                nc.vector.tensor_mul(ctx_sb[half:half + 64, pr, :],
                                     craw[0:HD, h01 * 1024:h01 * 1024 + 1024],
                                     recb[0:HD, h01 * 1024:h01 * 1024 + 1024])
        ps2c.release()
        ps2s.release()
        p_div1.release()
        p_div.release()
        p_attn.release()
        p_v.release()
        p_qk.release()

        # ------- Phase 3: proj + bias + residual, LN1 woven in by halves ----
        p_x1 = tc.alloc_tile_pool(name="p_x1", bufs=1)
        p_t = tc.alloc_tile_pool(name="p_t", bufs=2)
        ps_ln = tc.alloc_tile_pool(name="ps_ln", bufs=1, space="PSUM")
        ps_b = tc.alloc_tile_pool(name="ps_b", bufs=1, space="PSUM")
        ps3 = tc.alloc_tile_pool(name="ps3", bufs=4, space="PSUM")
        # r1 and y2 share one buffer: r1's last read (LN1 affine of half 1)
        # happens-before y2's first write (fc2 of half 0)
        r1_sb = p_x1.tile([128, DC, N], F32R, tag="big", name="r1")
        x116_sb = p_x1.tile([128, DC, N], F16, tag="x116")

        def proj_half(nb):
            sl = slice(nb * 512, nb * 512 + 512)
            for et in range(DC):
                ps = ps3.tile([128, 512], F32, tag="pj", name="pspj")
                for c in range(DC):
                    nc.tensor.matmul(ps[:], wproj_sb[:, c, et * 128:(et + 1) * 128],
                                     ctx_sb[:, c, sl],
                                     start=(c == 0), stop=(c == DC - 1))
                nc.vector.scalar_tensor_tensor(
                    out=r1_sb[:, et, sl], in0=ps[:],
                    scalar=bproj_sb[:, et:et + 1], in1=xT16_sb[:, et, sl],
                    op0=OP.add, op1=OP.add)

        proj_half(0)
        st0 = ln_stats(r1_sb, 0, p_t, ps_ln)
        proj_half(1)
        ln_affine(r1_sb, g1_sb, gb1_sb, 0, st0, x116_sb, F16, p_t, ps_ln, ps_b)
        st1 = ln_stats(r1_sb, 1, p_t, ps_ln)
        ps3.release()
        p_ctx.release()
        p_wproj.release()
        p_xT16.release()

        # ---------------- Phase 4: MLP + residual, LN2 by halves ------------
        y2_sb = p_x1.tile([128, DC, N], F32R, tag="big", name="y2")
        p_h = tc.alloc_tile_pool(name="p_h", bufs=2, side="right")
        ps4a = tc.alloc_tile_pool(name="ps4a", bufs=2, space="PSUM")
        ps4b = tc.alloc_tile_pool(name="ps4b", bufs=2, space="PSUM")

        def fc1_half(nb):
            sl = slice(nb * 512, nb * 512 + 512)
            hc = p_h.tile([128, FT, 512], F16, tag="h", name="hc")
            for ftg in range(FT):
                ps = ps4a.tile([128, 512], F32, tag="f1", name="psf1")
                for c in range(DC):
                    nc.tensor.matmul(ps[:], w1_sb[:, c, ftg * 128:(ftg + 1) * 128],
                                     x116_sb[:, c, sl],
                                     start=(c == 0), stop=(c == DC - 1))
                nc.scalar.activation(out=hc[:, ftg, :], in_=ps[:], func=AF.Gelu,
                                     bias=bfc1_sb[:, ftg:ftg + 1], scale=1.0)
            return hc

        def fc2_half(nb, hc):
            sl = slice(nb * 512, nb * 512 + 512)
            for et in range(DC):
                ps = ps4b.tile([128, 512], F32, tag="f2", name="psf2")
                for fc in range(FT):
                    nc.tensor.matmul(ps[:], w2_sb[:, fc, et * 128:(et + 1) * 128],
                                     hc[:, fc, :],
                                     start=(fc == 0), stop=(fc == FT - 1))
                nc.vector.scalar_tensor_tensor(
                    out=y2_sb[:, et, sl], in0=ps[:],
                    scalar=bfc2_sb[:, et:et + 1], in1=x116_sb[:, et, sl],
                    op0=OP.add, op1=OP.add)

        hc0 = fc1_half(0)
        ln_affine(r1_sb, g1_sb, gb1_sb, 1, st1, x116_sb, F16, p_t, ps_ln, ps_b)
        fc2_half(0, hc0)
        st20 = ln_stats(y2_sb, 0, p_t, ps_ln)
        hc1 = fc1_half(1)
        ln_affine(y2_sb, g2_sb, gb2_sb, 0, st20, None, F32, p_t, ps_ln, ps_b,
                  dma_out=yT)
        fc2_half(1, hc1)
        st21 = ln_stats(y2_sb, 1, p_t, ps_ln)
        ln_affine(y2_sb, g2_sb, gb2_sb, 1, st21, None, F32, p_t, ps_ln, ps_b,
                  dma_out=yT)
        ps4b.release()
        ps4a.release()
        p_h.release()
        ps_b.release()
        ps_ln.release()
        p_t.release()
        p_x1.release()
        p_w2.release()
        p_w1.release()
        dscr.release()
        stats.release()
        const.release()
    return nc


_NC_CACHE = None


def _get_nc():
    global _NC_CACHE
    if _NC_CACHE is None:
        nc = _build()
        _split_excess_waits(nc)
        _NC_CACHE = nc
    return _NC_CACHE


def kernel(x, w_qkv, w_proj, b_proj, w_fc1, b_fc1, w_fc2, b_fc2,
           gamma1, beta1, gamma2, beta2):
    global LAST_RESULT
    x = np.asarray(x, dtype=np.float32)
    w_qkv = np.asarray(w_qkv, dtype=np.float32)
    w_proj = np.asarray(w_proj, dtype=np.float32)
    b_proj = np.asarray(b_proj, dtype=np.float32)
    w_fc1 = np.asarray(w_fc1, dtype=np.float32)
    b_fc1 = np.asarray(b_fc1, dtype=np.float32)
    w_fc2 = np.asarray(w_fc2, dtype=np.float32)
    b_fc2 = np.asarray(b_fc2, dtype=np.float32)
    gamma1 = np.asarray(gamma1, dtype=np.float32)
    beta1 = np.asarray(beta1, dtype=np.float32)
    gamma2 = np.asarray(gamma2, dtype=np.float32)
    beta2 = np.asarray(beta2, dtype=np.float32)

    wqkv_scaled = w_qkv.copy()
    wqkv_scaled[:D] *= HD ** -0.5                  # fold attention scale into Q
    wqkvT = np.ascontiguousarray(wqkv_scaled.T.astype(np.float16))
    wprojT = np.ascontiguousarray(w_proj.T.astype(np.float16))
    wfc1T = np.ascontiguousarray(w_fc1.T.astype(np.float16))
    wfc2T = np.ascontiguousarray(w_fc2.T.astype(np.float16))

    def cols(v, nchunks):
        return np.ascontiguousarray(v.reshape(nchunks, 128).T)

    shared = {
        "wqkvT": wqkvT, "wprojT": wprojT, "wfc1T": wfc1T, "wfc2T": wfc2T,
        "bprojC": cols(b_proj, DC), "bfc1C": cols(b_fc1, FT),
        "bfc2C": cols(b_fc2, DC),
        "gamma1C": cols(gamma1, DC), "gamma2C": cols(gamma2, DC),
        "gb1R": np.ascontiguousarray(np.stack([gamma1, beta1])),
        "gb2R": np.ascontiguousarray(np.stack([gamma2, beta2])),
    }
    in_maps = []
    for b in range(NCORES):
        m = dict(shared)
        m["xT16"] = np.ascontiguousarray(x[b].T).astype(np.float16)
        in_maps.append(m)

    nc = _get_nc()
    LAST_RESULT = run_bass_kernel_spmd(nc, in_maps, list(range(NCORES)))
    out = np.stack([np.ascontiguousarray(LAST_RESULT.results[b]["yT"].T)
                    for b in range(NCORES)])
    return out.astype(np.float32)


# revision 25
# speedup vs baseline: 1.0362x; 1.0362x over previous
"""Trainium2 Bass kernel for a prenorm transformer Block (B=8, N=1024, D=768,
12 heads, MLP hidden 3072), data-parallel over batch across 8 NeuronCores.

Layout strategy: activations live transposed on-device — features on SBUF
partitions, tokens on the free dimension — so the whole chain
(QKV -> attention -> proj -> LN -> MLP -> LN) feeds the PE without any
on-device transposes:

  - qT/kT per head land as [64 dims (partitions), 1024 tokens]; scores are
    computed transposed (scoresT[m, n] = k_m . q_n) so softmax's exp is a
    plain ACT pass; the denominators come out of the attn@v matmul via an
    extra ones-column on the stationary V operand.
  - Softmax skips max-subtraction: scores here are bounded (|s| < ~4), exp
    cannot overflow fp32, and softmax is shift-invariant so results match.
  - The per-pair softmax division runs entirely off the PE critical path:
    the PSUM context tiles are copied to SBUF right away (freeing the PSUM
    banks for the next pair), then reciprocal_approx_fast + a DRAM-roundtrip
    partition-broadcast + one multiply produce ctx while the next pair's
    matmuls/exps stream. ctx is only consumed by proj at the end.
  - LayerNorm: sums and sum-of-squares reduce over features (partitions) on
    the PE as ones-vector matmuls; the per-token scale/shift expand to
    [128, 512] tiles as PE outer products against stationary gamma/beta
    rows, so the affine is 2 fused DVE ops per feature chunk. Squares run
    on the Scalar engine (tableless), 1/std via Sqrt + recip_approx_fast.
    Each LN is emitted as separate stats/affine halves woven between the
    surrounding matmul phases so the PE never drains.
  - MLP fc2 accumulates all 24 hidden chunks of a token half in PSUM
    (no vector-add accumulation); both residual adds are single fused
    scalar_tensor_tensor ops reading the PSUM result directly.
  - All matmuls use f16 weights / f16 or float32r moving operands
    (1 cycle/row on the PE); all weights are prefetched to SBUF during the
    ACT-bound attention phase.

Host side pre-transposes x and all weights, folds the attention scale into
the Q columns of w_qkv, and transposes the final output back.
"""
import sys
import types

sys.path.insert(0, "/opt/trn_rl_repo")

# concourse.bass_utils imports antenv.axon_hooks when tracing is requested;
# provide a no-op registry if the container image lacks that module so a
# BASS_TRACE=1 environment degrades to "no trace" instead of crashing.
try:
    import antenv.axon_hooks  # noqa: F401
except Exception:
    try:
        import antenv

        _hooks = types.ModuleType("antenv.axon_hooks")
        _hooks._hook = None

        def _set_hook(h):
            _hooks._hook = h

        def _get_hook():
            return _hooks._hook

        _hooks.set_axon_ntff_profile_hook = _set_hook
        _hooks.get_axon_ntff_profile_hook = _get_hook
        sys.modules["antenv.axon_hooks"] = _hooks
        antenv.axon_hooks = _hooks
    except Exception:
        pass

import numpy as np

import concourse.bass as bass
import concourse.tile as tile
from concourse import mybir
from concourse.bass_utils import run_bass_kernel_spmd

F32R = mybir.dt.float32r
F32 = mybir.dt.float32
F16 = mybir.dt.float16
AF = mybir.ActivationFunctionType
OP = mybir.AluOpType
I16 = mybir.dt.int16
I32 = mybir.dt.int32
K16 = 0x7799            # f16 reciprocal magic + 1  (seed = K - bits(d))
K32 = 0x7EF311C4        # f32 reciprocal magic + 1

NCORES = 8
D, HEADS, HID, N = 768, 12, 3072, 1024
HD = D // HEADS                  # 64 head dim
DC = D // 128                    # 6 feature chunks
NB = N // 512                    # 2 moving-dim blocks
MT = N // 128                    # 8 token tiles
FT = HID // 128                  # 24 hidden chunks
EPS = 1e-6

LAST_RESULT = None               # BassKernelResults of the most recent run


# The walrus build in this container rejects instructions carrying more than
# a couple of sync waits ("Too many sync wait commands"); self-loading fp32r
# matmuls reject more than one. Excess waits are hoisted onto standalone
# EventSemaphore carriers placed right before the instruction on the same
# engine, which is semantically identical (waits gate the engine stream).
_MM_OPS = ("Matmult", "Ldweights")


def _split_excess_waits(nc, default_limit=1, matmul_limit=0):
    counter = 0
    for f in nc.m.functions:
        for bb in f.blocks:
            new_insts = []
            for inst in bb.instructions:
                si = inst.sync_info
                waits = list(si.on_wait) if si and si.on_wait else []
                limit = matmul_limit if inst.opcode in _MM_OPS else default_limit
                if len(waits) > limit:
                    keep, move = waits[:limit], waits[limit:]
                    for w in move:
                        counter += 1
                        ev = mybir.InstEventSemaphore(
                            name=f"I-waitsplit-{counter}",
                            engine=inst.engine,
                            sync_info=mybir.SyncInfo(on_wait=[w], on_update=[]),
                        )
                        nc.register_instruction(ev, overwrite=True)
                        new_insts.append(ev)
                    inst.sync_info = mybir.SyncInfo(
                        on_wait=keep, on_update=list(si.on_update) if si else []
                    )
                new_insts.append(inst)
            bb.instructions = new_insts
    return counter


def _build():
    nc = bass.Bass()

    xT16 = nc.dram_tensor("xT16", [D, N], F16, kind="ExternalInput")
    wqkvT = nc.dram_tensor("wqkvT", [D, 3 * D], F16, kind="ExternalInput")
    wprojT = nc.dram_tensor("wprojT", [D, D], F16, kind="ExternalInput")
    wfc1T = nc.dram_tensor("wfc1T", [D, HID], F16, kind="ExternalInput")
    wfc2T = nc.dram_tensor("wfc2T", [HID, D], F16, kind="ExternalInput")
    bprojC = nc.dram_tensor("bprojC", [128, DC], F32, kind="ExternalInput")
    bfc1C = nc.dram_tensor("bfc1C", [128, FT], F32, kind="ExternalInput")
    bfc2C = nc.dram_tensor("bfc2C", [128, DC], F32, kind="ExternalInput")
    gamma1C = nc.dram_tensor("gamma1C", [128, DC], F32, kind="ExternalInput")
    gamma2C = nc.dram_tensor("gamma2C", [128, DC], F32, kind="ExternalInput")
    gb1R = nc.dram_tensor("gb1R", [2, D], F32, kind="ExternalInput")
    gb2R = nc.dram_tensor("gb2R", [2, D], F32, kind="ExternalInput")
    yT = nc.dram_tensor("yT", [D, N], F32, kind="ExternalOutput")

    with tile.TileContext(nc) as tc:
        const = tc.alloc_tile_pool(name="const", bufs=1)
        stats = tc.alloc_tile_pool(name="stats", bufs=1)
        dscr = tc.alloc_tile_pool(name="dscr", bufs=4, space="DRAM")

        ones128 = const.tile([128, 1], F32R)
        nc.vector.tensor_copy(ones128[:], nc.const_aps.tensor(1.0, (128, 1)))
        ones_row = const.tile([1, 128], F32R)
        nc.vector.tensor_copy(ones_row[:], nc.const_aps.tensor(1.0, (1, 128)))
        eps_t = const.tile([1, 1], F32)
        nc.vector.memset(eps_t[:], EPS)
        bproj_sb = const.tile([128, DC], F32)
        bfc1_sb = const.tile([128, FT], F32)
        bfc2_sb = const.tile([128, DC], F32)
        g1_sb = const.tile([128, DC], F32)
        g2_sb = const.tile([128, DC], F32)
        gbf_sb = const.tile([2, D], F32)
        gb1_sb = const.tile([2, D], F16)
        gb2_sb = const.tile([2, D], F16)
        for t, src in ((bproj_sb, bprojC), (bfc1_sb, bfc1C), (bfc2_sb, bfc2C),
                       (g1_sb, gamma1C), (g2_sb, gamma2C)):
            nc.sync.dma_start(out=t[:], in_=src[:])
        nc.sync.dma_start(out=gbf_sb[:], in_=gb1R[:])
        nc.vector.tensor_copy(gb1_sb[:], gbf_sb[:])
        nc.sync.dma_start(out=gbf_sb[:], in_=gb2R[:])
        nc.vector.tensor_copy(gb2_sb[:], gbf_sb[:])
        # moving operand for the LN shift outer-product: row0 = -mu/std
        # (written per LN half), row1 = ones
        m2 = [const.tile([2, 512], F16, name=f"m2_{nb}") for nb in range(NB)]
        for t in m2:
            nc.vector.memset(t[:], 1.0)   # row 0 is rewritten per LN half

        def ln_stats(src_sb, nb, p_t, ps_ln):
            """Per-token 1/std (r) and -mu/std (m2[nb] row 0) for token half
            nb of src_sb [128, DC, N] f32r. Reductions over features run on
            the PE; the small chains on ACT/DVE."""
            sl = slice(nb * 512, nb * 512 + 512)
            s1 = ps_ln.tile([1, 512], F32, tag="s1", name="s1")
            s2 = ps_ln.tile([1, 512], F32, tag="s2", name="s2")
            for c in range(DC):
                nc.tensor.matmul(s1[:], ones128[:], src_sb[:, c, sl],
                                 start=(c == 0), stop=(c == DC - 1))
            for c in range(DC):
                sq = p_t.tile([128, 512], F32R, tag="sq", name="sq")
                nc.scalar.activation(out=sq[:], in_=src_sb[:, c, sl].bitcast(F32),
                                     func=AF.Square)
                nc.tensor.matmul(s2[:], ones128[:], sq[:],
                                 start=(c == 0), stop=(c == DC - 1))
            u = stats.tile([1, 512], F32, tag=f"u{nb}", name="u")
            w = stats.tile([1, 512], F32, tag=f"w{nb}", name="w")
            sd = stats.tile([1, 512], F32R, tag=f"sd{nb}", name="sd")
            nc.scalar.activation(out=u[:], in_=s1[:], func=AF.Square)
            nc.vector.scalar_tensor_tensor(out=w[:], in0=s2[:], scalar=float(D),
                                           in1=u[:], op0=OP.mult, op1=OP.subtract)
            nc.scalar.activation(out=sd[:], in_=w[:], func=AF.Sqrt,
                                 bias=eps_t[:], scale=1.0 / (D * D))   # std, f32r
            return s1, sd

        def ln_affine(src_sb, gamcol, gbrows, nb, stats_t, out_full, out_dt,
                      p_t, ps_ln, ps_b, dma_out=None):
            """out = src * gamma * bcast(1/std) + (gamma (x) m2row0 + beta).
            1/std via magic-seed + 2 Newton steps on DVE (this walrus build
            rejects custom-DVE and ACT Rsqrt/Reciprocal); the broadcast is a
            PE outer product; per chunk one fused STT plus one TT add on
            DVE. out_full is a [128, DC, N] tile or None (then per-chunk
            ring tiles are DMAed straight out)."""
            s1, sd = stats_t
            sl = slice(nb * 512, nb * 512 + 512)
            na = stats.tile([1, 512], F32, tag="na", name="na")
            nb_ = stats.tile([1, 512], F32, tag="nb", name="nb")
            ng = stats.tile([1, 512], F32, tag="ng", name="ng")
            rT = stats.tile([1, 512], F32R, tag="rT", name="rT")
            sdf = sd[:].bitcast(F32)
            nc.vector.tensor_scalar(out=na[:].bitcast(I32), in0=sd[:].bitcast(I32),
                                    scalar1=-1, scalar2=None, op0=OP.bitwise_xor)
            nc.vector.tensor_scalar(out=nb_[:].bitcast(I32), in0=na[:].bitcast(I32),
                                    scalar1=K32, scalar2=None, op0=OP.add)
            nc.vector.tensor_mul(na[:], sdf, nb_[:])
            nc.vector.tensor_scalar(out=ng[:], in0=na[:], scalar1=-1.0,
                                    scalar2=2.0, op0=OP.mult, op1=OP.add)
            nc.vector.tensor_mul(na[:], nb_[:], ng[:])        # y1
            nc.vector.tensor_mul(nb_[:], sdf, na[:])
            nc.vector.tensor_scalar(out=ng[:], in0=nb_[:], scalar1=-1.0,
                                    scalar2=2.0, op0=OP.mult, op1=OP.add)
            nc.vector.tensor_mul(rT[:], na[:], ng[:])         # 1/std, f32r
            nc.vector.scalar_tensor_tensor(out=m2[nb][0:1, :], in0=s1[:],
                                           scalar=-1.0 / D, in1=rT[:].bitcast(F32),
                                           op0=OP.mult, op1=OP.mult)  # -mu/std
            R = ps_ln.tile([128, 512], F32, tag="R", name="R")
            nc.tensor.matmul(R[:], ones_row[:], rT[:],
                             start=True, stop=True)          # bcast 1/std
            for c in range(DC):
                B = ps_b.tile([128, 512], F32, tag="B", name="B")
                nc.tensor.matmul(B[:], gbrows[:, c * 128:(c + 1) * 128],
                                 m2[nb][:], start=True, stop=True)
                t = p_t.tile([128, 512], F32, tag="t", name="t")
                nc.vector.scalar_tensor_tensor(
                    out=t[:], in0=src_sb[:, c, sl].bitcast(F32),
                    scalar=gamcol[:, c:c + 1], in1=R[:],
                    op0=OP.mult, op1=OP.mult)
                if out_full is not None:
                    nc.vector.tensor_add(out_full[:, c, sl], t[:], B[:])
                else:
                    o = p_t.tile([128, 512], out_dt, tag="o", name="o")
                    nc.vector.tensor_add(o[:], t[:], B[:])
                    nc.sync.dma_start(out=dma_out[c * 128:(c + 1) * 128, sl],
                                      in_=o[:])

        # ---------------- Phase 1: QKV projections ----------------
        p_w1 = tc.alloc_tile_pool(name="p_w1", bufs=1)
        w1_sb = p_w1.tile([128, DC, HID], F16)
        p_w2 = tc.alloc_tile_pool(name="p_w2", bufs=1)
        w2_sb = p_w2.tile([128, FT, D], F16)
        p_xT16 = tc.alloc_tile_pool(name="p_xT16", bufs=1, side="right")
        xT16_sb = p_xT16.tile([128, DC, N], F16)
        def dma_x16(nb):
            sl = slice(nb * 512, nb * 512 + 512)
            for c in range(DC):
                nc.sync.dma_start(out=xT16_sb[:, c, sl],
                                  in_=xT16[c * 128:(c + 1) * 128, sl])
        dma_x16(0)
        p_wproj = tc.alloc_tile_pool(name="p_wproj", bufs=1, side="right")
        wproj_sb = p_wproj.tile([128, DC, D], F16)
        p_ctx = tc.alloc_tile_pool(name="p_ctx", bufs=1, side="right")
        ctx_sb = p_ctx.tile([128, DC, N], F16)
        p_qk = tc.alloc_tile_pool(name="p_qk", bufs=1, side="right")
        p_v = tc.alloc_tile_pool(name="p_v", bufs=1, side="right")
        p_attn = tc.alloc_tile_pool(name="p_attn", bufs=4, side="right")
        p_wqkv = tc.alloc_tile_pool(name="p_wqkv", bufs=1, side="right")
        wqkv_sb = p_wqkv.tile([128, DC, 3 * D], F16)
        # weight columns arrive in matmul order: (q_i, k_i) pairs first
        # fine-grained, then the second x half, then v as one block
        def dma_wcols(jts):
            for jt in jts:
                for c in range(DC):
                    cs = slice(jt * 128, (jt + 1) * 128)
                    nc.sync.dma_start(out=wqkv_sb[:, c, cs],
                                      in_=wqkvT[c * 128:(c + 1) * 128, cs])
        dma_wcols([0, DC])
        dma_x16(1)
        dma_wcols([1, DC + 1, 2, DC + 2])
        for c in range(DC):
            nc.sync.dma_start(out=wqkv_sb[:, c, 3 * 128:DC * 128],
                              in_=wqkvT[c * 128:(c + 1) * 128, 3 * 128:DC * 128])
        for c in range(DC):
            nc.sync.dma_start(out=wqkv_sb[:, c, (DC + 3) * 128:2 * D],
                              in_=wqkvT[c * 128:(c + 1) * 128, (DC + 3) * 128:2 * D])
        for c in range(DC):
            nc.sync.dma_start(out=wqkv_sb[:, c, 2 * D:3 * D],
                              in_=wqkvT[c * 128:(c + 1) * 128, 2 * D:3 * D])
        q_sb = p_qk.tile([128, DC, N], F16)
        k_sb = p_qk.tile([128, DC, N], F16)
        v_sb = p_v.tile([128, MT, HEADS, HD + 1], F16)
        nc.vector.tensor_copy(v_sb[:, :, :, HD:HD + 1],
                              nc.const_aps.tensor(1.0, (128, MT, HEADS, 1)))

        ps2s = tc.alloc_tile_pool(name="ps2s", bufs=1, space="PSUM")
        ps1 = tc.alloc_tile_pool(name="ps1", bufs=2, space="PSUM")
        ps1v = tc.alloc_tile_pool(name="ps1v", bufs=1, space="PSUM")
        # warm the Exp activation table while QKV streams, so the first
        # attention exp pays no table load
        warm = stats.tile([1, 64], F32, tag="warm", name="warm")
        nc.scalar.activation(out=warm[:], in_=warm[:], func=AF.Exp)

        def qk_chain(jt, nb):
            sl = slice(nb * 512, nb * 512 + 512)
            ps = ps1.tile([128, 512], F32, tag="qk", name="psqk")
            for c in range(DC):
                nc.tensor.matmul(ps[:], wqkv_sb[:, c, jt * 128:(jt + 1) * 128],
                                 xT16_sb[:, c, sl],
                                 start=(c == 0), stop=(c == DC - 1))
            dst = q_sb if jt < DC else k_sb
            nc.scalar.activation(out=dst[:, jt % DC, sl], in_=ps[:],
                                 func=AF.Copy, scale=1.0)

        def v_chain(mt):
            ps = ps1v.tile([128, D], F32, tag="v", name="psv")
            for c in range(DC):
                nc.tensor.matmul(ps[:, 0:512],
                                 xT16_sb[:, c, mt * 128:(mt + 1) * 128],
                                 wqkv_sb[:, c, 2 * D:2 * D + 512],
                                 start=(c == 0), stop=(c == DC - 1))
                nc.tensor.matmul(ps[:, 512:768],
                                 xT16_sb[:, c, mt * 128:(mt + 1) * 128],
                                 wqkv_sb[:, c, 2 * D + 512:3 * D],
                                 start=(c == 0), stop=(c == DC - 1))
            nc.vector.tensor_copy(v_sb[:, mt, :, 0:HD],
                                  ps[:].rearrange("p (h d) -> p h d", h=HEADS))

        def scores_mt(pr, mt):
            pse = ps2s.tile([128, N], F32, tag="se", name="pse")
            pso = ps2s.tile([128, N], F32, tag="so", name="pso")
            msl = slice(mt * 128, mt * 128 + 128)
            for nb in range(NB):
                sl = slice(nb * 512, nb * 512 + 512)
                nc.tensor.matmul(pse[:, sl], k_sb[0:64, pr, msl],
                                 q_sb[0:64, pr, sl], start=True, stop=True)
                nc.tensor.matmul(pso[:, sl], k_sb[64:128, pr, msl],
                                 q_sb[64:128, pr, sl], start=True, stop=True)
            ae = p_attn.tile([128, N], F16, tag="attnT", name="ae")
            ao = p_attn.tile([128, N], F16, tag="attnT", name="ao")
            nc.scalar.activation(out=ae[:], in_=pse[:], func=AF.Exp)
            nc.scalar.activation(out=ao[:], in_=pso[:], func=AF.Exp)
            return ae, ao

        for jt in (0, DC):
            qk_chain(jt, 0)
        for jt in (0, DC):
            qk_chain(jt, 1)
        for pair in range(1, DC):
            qk_chain(pair, 0)
            qk_chain(DC + pair, 0)
            qk_chain(pair, 1)
            qk_chain(DC + pair, 1)
        # prerun pair-0 scores/exps between v chains: the ACT pipeline gets a
        # head start and the PE has no idle seam entering attention
        v_chain(0)
        pre = [scores_mt(0, 0)]
        for mt in range(1, MT):
            v_chain(mt)
            if mt == 1:
                pre.append(scores_mt(0, 1))
        ps1v.release()
        ps1.release()
        p_wqkv.release()
        p_div = tc.alloc_tile_pool(name="p_div", bufs=2, side="right")
        p_div1 = tc.alloc_tile_pool(name="p_div1", bufs=1, side="right")

        # proj/MLP weights: DMA during the (ACT-bound) attention phase
        for c in range(DC):
            nc.sync.dma_start(out=wproj_sb[:, c, :], in_=wprojT[c * 128:(c + 1) * 128, :])
        for c in range(DC):
            nc.sync.dma_start(out=w1_sb[:, c, :], in_=wfc1T[c * 128:(c + 1) * 128, :])
        for fc in range(FT):
            nc.sync.dma_start(out=w2_sb[:, fc, :], in_=wfc2T[fc * 128:(fc + 1) * 128, :])

        # ---------------- Phase 2: attention (head pairs) ----------------
        ps2c = tc.alloc_tile_pool(name="ps2c", bufs=1, space="PSUM")

        for pr in range(HEADS // 2):
            cps = {}
            for j in range(4):          # j = h01*2 + nb
                cps[j] = ps2c.tile([HD + 1, 512], F32, tag=f"c{j}", name=f"cps{j}")
            for mt in range(MT):
                if pr == 0 and mt < len(pre):
                    ae, ao = pre[mt]
                else:
                    ae, ao = scores_mt(pr, mt)
                for h01, at_t in ((0, ae), (1, ao)):
                    h = 2 * pr + h01
                    for nb in range(NB):
                        sl = slice(nb * 512, nb * 512 + 512)
                        nc.tensor.matmul(cps[2 * h01 + nb][:], v_sb[:, mt, h, :],
                                         at_t[:, sl],
                                         start=(mt == 0), stop=(mt == MT - 1))
            # softmax division — the f16 copies free the cps PSUM banks for
            # the next pair; the Newton reciprocal + DRAM-roundtrip
            # broadcast + multiply are off the PE critical path (ctx is only
            # consumed by proj after the last pair).
            craw = p_div.tile([HD, 4 * 512], F16, tag="craw", name="craw")
            dnt = p_div1.tile([1, 4 * 512], F16, tag="dn", name="dnt")
            for j in range(4):
                nc.vector.tensor_copy(craw[:, j * 512:(j + 1) * 512],
                                      cps[j][0:HD, :])
                nc.vector.tensor_copy(dnt[0:1, j * 512:(j + 1) * 512],
                                      cps[j][HD:HD + 1, :])
            dn = dnt[:]
            a = p_div1.tile([1, 4 * 512], F16, tag="a", name="na")
            b = p_div1.tile([1, 4 * 512], F16, tag="b", name="nb")
            g = p_div1.tile([1, 4 * 512], F16, tag="g", name="ng")
            nc.vector.tensor_scalar(out=a[:].bitcast(I16), in0=dn.bitcast(I16),
                                    scalar1=-1, scalar2=None, op0=OP.bitwise_xor)
            nc.vector.tensor_scalar(out=b[:].bitcast(I16), in0=a[:].bitcast(I16),
                                    scalar1=K16, scalar2=None, op0=OP.add)
            nc.vector.tensor_mul(a[:], dn, b[:])
            nc.vector.tensor_scalar(out=g[:], in0=a[:], scalar1=-1.0,
                                    scalar2=2.0, op0=OP.mult, op1=OP.add)
            nc.vector.tensor_mul(a[:], b[:], g[:])            # y1
            if pr < HEADS // 2 - 1:
                nc.vector.tensor_mul(b[:], dn, a[:])
                nc.vector.tensor_scalar(out=g[:], in0=b[:], scalar1=-1.0,
                                        scalar2=2.0, op0=OP.mult, op1=OP.add)
                nc.vector.tensor_mul(b[:], a[:], g[:])        # 1/denom
            else:
                # last pair: 1 Newton step (0.26% worst case) — this division
                # gates proj, shave its latency
                a, b = b, a
            scr = dscr.tile([4 * 512], F16, name="bscr")
            nc.sync.dma_start(out=scr[:], in_=b[:])
            recb = p_div1.tile([128, 4 * 512], F16, tag="recb", name="recb")
            nc.sync.dma_start(out=recb[:],
                              in_=scr[:].unsqueeze(0).to_broadcast([128, 4 * 512]))
            for h01 in range(2):
                half = h01 * 64
                nc.vector.tensor_mul(ctx_sb[half:half + 64, pr, :],
                                     craw[0:HD, h01 * 1024:h01 * 1024 + 1024],
                                     recb[0:HD, h01 * 1024:h01 * 1024 + 1024])
        ps2c.release()
        ps2s.release()
        p_div1.release()
        p_div.release()
        p_attn.release()
        p_v.release()
        p_qk.release()

        # ------- Phase 3: proj + bias + residual, LN1 woven in by halves ----
        p_x1 = tc.alloc_tile_pool(name="p_x1", bufs=1)
        p_t = tc.alloc_tile_pool(name="p_t", bufs=2)
        ps_ln = tc.alloc_tile_pool(name="ps_ln", bufs=1, space="PSUM")
        ps_b = tc.alloc_tile_pool(name="ps_b", bufs=1, space="PSUM")
        ps3 = tc.alloc_tile_pool(name="ps3", bufs=4, space="PSUM")
        # r1 and y2 share one buffer: r1's last read (LN1 affine of half 1)
        # happens-before y2's first write (fc2 of half 0)
        r1_sb = p_x1.tile([128, DC, N], F32R, tag="big", name="r1")
        x116_sb = p_x1.tile([128, DC, N], F16, tag="x116")

        def proj_half(nb):
            sl = slice(nb * 512, nb * 512 + 512)
            for et in range(DC):
                ps = ps3.tile([128, 512], F32, tag="pj", name="pspj")
                for c in range(DC):
                    nc.tensor.matmul(ps[:], wproj_sb[:, c, et * 128:(et + 1) * 128],
                                     ctx_sb[:, c, sl],
                                     start=(c == 0), stop=(c == DC - 1))
                t = p_t.tile([128, 512], F32, tag="ep", name="ep")
                nc.scalar.activation(out=t[:], in_=ps[:], func=AF.Identity,
                                     bias=bproj_sb[:, et:et + 1], scale=1.0)
                nc.gpsimd.tensor_tensor(out=r1_sb[:, et, sl], in0=t[:],
                                        in1=xT16_sb[:, et, sl], op=OP.add)

        proj_half(0)
        st0 = ln_stats(r1_sb, 0, p_t, ps_ln)
        proj_half(1)
        ln_affine(r1_sb, g1_sb, gb1_sb, 0, st0, x116_sb, F16, p_t, ps_ln, ps_b)
        st1 = ln_stats(r1_sb, 1, p_t, ps_ln)
        ps3.release()
        p_ctx.release()
        p_wproj.release()
        p_xT16.release()

        # ---------------- Phase 4: MLP + residual, LN2 by halves ------------
        y2_sb = p_x1.tile([128, DC, N], F32R, tag="big", name="y2")
        p_h = tc.alloc_tile_pool(name="p_h", bufs=2, side="right")
        ps4a = tc.alloc_tile_pool(name="ps4a", bufs=2, space="PSUM")
        ps4b = tc.alloc_tile_pool(name="ps4b", bufs=2, space="PSUM")

        def fc1_half(nb):
            sl = slice(nb * 512, nb * 512 + 512)
            hc = p_h.tile([128, FT, 512], F16, tag="h", name="hc")
            for ftg in range(FT):
                ps = ps4a.tile([128, 512], F32, tag="f1", name="psf1")
                for c in range(DC):
                    nc.tensor.matmul(ps[:], w1_sb[:, c, ftg * 128:(ftg + 1) * 128],
                                     x116_sb[:, c, sl],
                                     start=(c == 0), stop=(c == DC - 1))
                nc.scalar.activation(out=hc[:, ftg, :], in_=ps[:], func=AF.Gelu,
                                     bias=bfc1_sb[:, ftg:ftg + 1], scale=1.0)
            return hc

        def fc2_half(nb, hc):
            sl = slice(nb * 512, nb * 512 + 512)
            for et in range(DC):
                ps = ps4b.tile([128, 512], F32, tag="f2", name="psf2")
                for fc in range(FT):
                    nc.tensor.matmul(ps[:], w2_sb[:, fc, et * 128:(et + 1) * 128],
                                     hc[:, fc, :],
                                     start=(fc == 0), stop=(fc == FT - 1))
                t = p_t.tile([128, 512], F32, tag="ep", name="ep")
                nc.scalar.activation(out=t[:], in_=ps[:], func=AF.Identity,
                                     bias=bfc2_sb[:, et:et + 1], scale=1.0)
                nc.gpsimd.tensor_tensor(out=y2_sb[:, et, sl], in0=t[:],
                                        in1=x116_sb[:, et, sl], op=OP.add)

        hc0 = fc1_half(0)
        ln_affine(r1_sb, g1_sb, gb1_sb, 1, st1, x116_sb, F16, p_t, ps_ln, ps_b)
        fc2_half(0, hc0)
        st20 = ln_stats(y2_sb, 0, p_t, ps_ln)
        hc1 = fc1_half(1)
        ln_affine(y2_sb, g2_sb, gb2_sb, 0, st20, None, F32, p_t, ps_ln, ps_b,
                  dma_out=yT)
        fc2_half(1, hc1)
        st21 = ln_stats(y2_sb, 1, p_t, ps_ln)
        ln_affine(y2_sb, g2_sb, gb2_sb, 1, st21, None, F32, p_t, ps_ln, ps_b,
                  dma_out=yT)
        ps4b.release()
        ps4a.release()
        p_h.release()
        ps_b.release()
        ps_ln.release()
        p_t.release()
        p_x1.release()
        p_w2.release()
        p_w1.release()
        dscr.release()
        stats.release()
        const.release()
    return nc


_NC_CACHE = None


def _get_nc():
    global _NC_CACHE
    if _NC_CACHE is None:
        nc = _build()
        _split_excess_waits(nc)
        _NC_CACHE = nc
    return _NC_CACHE


def kernel(x, w_qkv, w_proj, b_proj, w_fc1, b_fc1, w_fc2, b_fc2,
           gamma1, beta1, gamma2, beta2):
    global LAST_RESULT
    x = np.asarray(x, dtype=np.float32)
    w_qkv = np.asarray(w_qkv, dtype=np.float32)
    w_proj = np.asarray(w_proj, dtype=np.float32)
    b_proj = np.asarray(b_proj, dtype=np.float32)
    w_fc1 = np.asarray(w_fc1, dtype=np.float32)
    b_fc1 = np.asarray(b_fc1, dtype=np.float32)
    w_fc2 = np.asarray(w_fc2, dtype=np.float32)
    b_fc2 = np.asarray(b_fc2, dtype=np.float32)
    gamma1 = np.asarray(gamma1, dtype=np.float32)
    beta1 = np.asarray(beta1, dtype=np.float32)
    gamma2 = np.asarray(gamma2, dtype=np.float32)
    beta2 = np.asarray(beta2, dtype=np.float32)

    wqkv_scaled = w_qkv.copy()
    wqkv_scaled[:D] *= HD ** -0.5                  # fold attention scale into Q
    wqkvT = np.ascontiguousarray(wqkv_scaled.T.astype(np.float16))
    wprojT = np.ascontiguousarray(w_proj.T.astype(np.float16))
    wfc1T = np.ascontiguousarray(w_fc1.T.astype(np.float16))
    wfc2T = np.ascontiguousarray(w_fc2.T.astype(np.float16))

    def cols(v, nchunks):
        return np.ascontiguousarray(v.reshape(nchunks, 128).T)

    shared = {
        "wqkvT": wqkvT, "wprojT": wprojT, "wfc1T": wfc1T, "wfc2T": wfc2T,
        "bprojC": cols(b_proj, DC), "bfc1C": cols(b_fc1, FT),
        "bfc2C": cols(b_fc2, DC),
        "gamma1C": cols(gamma1, DC), "gamma2C": cols(gamma2, DC),
        "gb1R": np.ascontiguousarray(np.stack([gamma1, beta1])),
        "gb2R": np.ascontiguousarray(np.stack([gamma2, beta2])),
    }
    in_maps = []
    for b in range(NCORES):
        m = dict(shared)
        m["xT16"] = np.ascontiguousarray(x[b].T).astype(np.float16)
        in_maps.append(m)

    nc = _get_nc()
    LAST_RESULT = run_bass_kernel_spmd(nc, in_maps, list(range(NCORES)))
    out = np.stack([np.ascontiguousarray(LAST_RESULT.results[b]["yT"].T)
                    for b in range(NCORES)])
    return out.astype(np.float32)
